# revision 1
# baseline (speedup 1.0000x reference)
"""HAN (2 meta-paths x 8 GAT heads) Trainium2 kernel, 8-core SPMD.

Strategy:
 - Host: sort each meta-path's edges by src, shard by src-range across 8 cores
   (6250 nodes/core, padded to 6272 = 49*128). Within a core, edges grouped in
   128-node windows, each padded to a uniform CALLS*128 edges (dummy edges
   point at a table row whose s_dst is huge -> weight exp(-lrelu(big)) = 0).
 - Device, per core (identical SPMD program):
   Phase T: table G[p] = [h(512) | s_dst(8) | s_src(8)] = xT.T @ Waug for ALL
            nodes (replicated compute; x fed pre-transposed from host).
   Phase S: s_src for the core's own 6272 nodes from its xTc slice.
   Phase E: per window: build selection matrices S/ST from src ids via
            iota-compare; per 128-edge call: indirect-DMA gather of G rows by
            dst, edge weights w = exp(-lrelu(s_src[src]+s_dst[dst])) with
            s_src[src] done as S.T @ s_src_window matmul, weighted rows
            ST.T @ (w * h_dst) accumulated in PSUM -> z = elu(num/den),
            transposed to feature-major zT; partial semantic scores
            tanh(z@Wp+bp)@q accumulated.
   Phase W: AllReduce the semantic score sums -> beta = softmax(mean).
   Phase F: out = sigmoid((beta0*z0 + beta1*z1) @ Wc), feature-major,
            written as [8, 6272] (host transposes / trims).
"""

import numpy as np

import concourse.bass as bass
import concourse.tile as tile
from concourse import bacc, mybir
from concourse.bass import IndirectOffsetOnAxis
from concourse.bass_utils import run_bass_kernel_spmd
from concourse.masks import make_identity

F32 = mybir.dt.float32
I32 = mybir.dt.int32


def _apx(ap, *dims):
    """AP with the source's partition dim replaced/kept and explicit free dims.

    dims[0] is the partition [step, count]; pass None to keep the source's.
    Remaining entries are [step, count] free dims (step in elements).
    """
    p = list(ap.ap[0]) if dims[0] is None else list(dims[0])
    return bass.AP(ap.tensor, ap.offset, [p] + [list(d) for d in dims[1:]])

# Model dims (fixed by the problem)
N, E = 50000, 1600000
NFEAT, NHID, NHEADS, NSEM, NMP, NLABEL = 256, 64, 8, 2, 128, 8
ALPHA = 0.2
D = NHID * NHEADS          # 512
TC = D + NHEADS + NHEADS   # 528 table cols: h | s_dst | s_src

NCORES = 8
NPC = N // NCORES          # 6250 nodes per core
NWIN = (NPC + 127) // 128  # 49
NPC_PAD = NWIN * 128       # 6272
NPAD = ((N + 1 + 127) // 128) * 128  # 50176 table rows (incl dummy)
DUMMY = NPAD - 1
NT_TILES = NPAD // 128     # 392
BIG = 1.0e9                # s_dst of dummy row -> weight 0
CB = 5                     # calls per fused DVE sub-block


# ---------------------------------------------------------------- program ---
def build_program(CALLS):
    nc = bacc.Bacc("TRN2", target_bir_lowering=False, debug=False,
                   num_devices=NCORES)

    # I/O
    xT = nc.dram_tensor("xT", [NFEAT, NPAD], F32, kind="ExternalInput").ap()
    xTc = nc.dram_tensor("xTc", [NFEAT, NPC_PAD], F32, kind="ExternalInput").ap()
    Waug = nc.dram_tensor("Waug", [NSEM, NFEAT, TC], F32, kind="ExternalInput").ap()
    gidx = nc.dram_tensor("gidx", [NSEM, NWIN, 128, CALLS], I32, kind="ExternalInput").ap()
    scol = nc.dram_tensor("scol", [NSEM, NWIN, 128, CALLS], F32, kind="ExternalInput").ap()
    srow = nc.dram_tensor("srow", [NSEM, NWIN, CALLS * 128], F32, kind="ExternalInput").ap()
    Wp = nc.dram_tensor("Wp", [D, NMP], F32, kind="ExternalInput").ap()
    bp = nc.dram_tensor("bp", [NMP, 1], F32, kind="ExternalInput").ap()
    qv = nc.dram_tensor("qv", [NMP, 1], F32, kind="ExternalInput").ap()
    Wc = nc.dram_tensor("Wc", [D, NLABEL], F32, kind="ExternalInput").ap()
    wbias = nc.dram_tensor("wbias", [1, NSEM], F32, kind="ExternalInput").ap()
    outT = nc.dram_tensor("outT", [NLABEL, NPC_PAD], F32, kind="ExternalOutput").ap()

    # internal DRAM
    G = [nc.dram_tensor(f"G{p}", [NPAD, TC], F32).ap() for p in range(NSEM)]
    ssrc = nc.dram_tensor("ssrc", [NSEM, NPC_PAD, NHEADS], F32).ap()
    zT = nc.dram_tensor("zT", [NSEM, 4, 128, NPC_PAD], F32).ap()
    wsin = nc.dram_tensor("wsin", [1, NSEM], F32).ap()
    wsout = nc.dram_tensor("wsout", [1, NSEM], F32, addr_space="Shared").ap()

    with tile.TileContext(nc) as tc:
        # ---------------- Phase T: replicated table build --------------------
        with tc.tile_pool(name="t_w", bufs=1) as wpool, \
             tc.tile_pool(name="t_x", bufs=3) as xpool, \
             tc.tile_pool(name="t_g", bufs=3) as gpool, \
             tc.tile_pool(name="t_ps", bufs=2, space="PSUM") as pspool:
            wa = []
            for p in range(NSEM):
                w0 = wpool.tile([128, TC], F32, tag=f"wa{p}0")
                w1 = wpool.tile([128, TC], F32, tag=f"wa{p}1")
                nc.sync.dma_start(w0[:], Waug[p, 0:128, :])
                nc.sync.dma_start(w1[:], Waug[p, 128:256, :])
                wa.append((w0, w1))
            for p in range(NSEM):
                w0, w1 = wa[p]
                for nt in range(NT_TILES):
                    r0 = nt * 128
                    x0 = xpool.tile([128, 128], F32, tag="x0")
                    x1 = xpool.tile([128, 128], F32, tag="x1")
                    nc.sync.dma_start(x0[:], xT[0:128, r0:r0 + 128])
                    nc.sync.dma_start(x1[:], xT[128:256, r0:r0 + 128])
                    psA = pspool.tile([128, D], F32, tag="psA")
                    psB = pspool.tile([128, 16], F32, tag="psB")
                    nc.tensor.matmul(psA[:], lhsT=x0[:], rhs=w0[:, 0:D], start=True, stop=False)
                    nc.tensor.matmul(psA[:], lhsT=x1[:], rhs=w1[:, 0:D], start=False, stop=True)
                    nc.tensor.matmul(psB[:], lhsT=x0[:], rhs=w0[:, D:TC], start=True, stop=False)
                    nc.tensor.matmul(psB[:], lhsT=x1[:], rhs=w1[:, D:TC], start=False, stop=True)
                    gt = gpool.tile([128, TC], F32, tag="gt")
                    nc.vector.tensor_copy(gt[:, 0:D], psA[:])
                    nc.vector.tensor_copy(gt[:, D:TC], psB[:])
                    nc.sync.dma_start(G[p][r0:r0 + 128, :], gt[:])
            # ---------------- Phase S: per-core local s_src -------------------
            for p in range(NSEM):
                w0, w1 = wa[p]
                for w in range(NWIN):
                    r0 = w * 128
                    x0 = xpool.tile([128, 128], F32, tag="x0")
                    x1 = xpool.tile([128, 128], F32, tag="x1")
                    nc.sync.dma_start(x0[:], xTc[0:128, r0:r0 + 128])
                    nc.sync.dma_start(x1[:], xTc[128:256, r0:r0 + 128])
                    psB = pspool.tile([128, 16], F32, tag="psB")
                    nc.tensor.matmul(psB[:, 0:8], lhsT=x0[:], rhs=w0[:, D + 8:TC], start=True, stop=False)
                    nc.tensor.matmul(psB[:, 0:8], lhsT=x1[:], rhs=w1[:, D + 8:TC], start=False, stop=True)
                    st = gpool.tile([128, 8], F32, tag="st")
                    nc.vector.tensor_copy(st[:], psB[:, 0:8])
                    nc.sync.dma_start(ssrc[p, r0:r0 + 128, :], st[:])
            # dummy row: s_dst = BIG so padded edges get weight exp(-BIG)=0
            dt_ = gpool.tile([1, 8], F32, tag="dum")
            nc.vector.memset(dt_[:], BIG)
            nc.sync.dma_start(G[0][DUMMY:DUMMY + 1, D:D + 8], dt_[:])
            nc.sync.dma_start(G[1][DUMMY:DUMMY + 1, D:D + 8], dt_[:])

        tc.strict_bb_all_engine_barrier()

        # ---------------- Phase E: edge gather + segment sums ----------------
        NBLK = (CALLS + CB - 1) // CB
        with tc.tile_pool(name="e_const", bufs=1) as cpool, \
             tc.tile_pool(name="e_z", bufs=2) as zpool, \
             tc.tile_pool(name="e_acc", bufs=1) as accpool:

            ident = cpool.tile([128, 128], F32, tag="ident")
            make_identity(nc, ident[:])
            icol = cpool.tile([128, 1], I32, tag="icol")
            nc.gpsimd.iota(icol[:], pattern=[[0, 1]], base=0, channel_multiplier=1)
            irow = cpool.tile([128, 128], I32, tag="irow")
            nc.gpsimd.iota(irow[:], pattern=[[1, 128]], base=0, channel_multiplier=0)
            wp_sb = cpool.tile([128, NMP * 4], F32, tag="wp")
            for k in range(4):
                nc.sync.dma_start(wp_sb[:, k * NMP:(k + 1) * NMP], Wp[k * 128:(k + 1) * 128, :])
            bp_sb = cpool.tile([128, 1], F32, tag="bp")
            nc.sync.dma_start(bp_sb[:], bp[:, :])
            q_sb = cpool.tile([128, 1], F32, tag="q")
            nc.sync.dma_start(q_sb[:], qv[:, :])
            wacc = []
            for p in range(NSEM):
                wt = accpool.tile([1, 128], F32, tag=f"wacc{p}")
                nc.vector.memset(wt[:], 0.0)
                wacc.append(wt)

            with tc.tile_pool(name="e_stage", bufs=2) as stpool, \
                 tc.tile_pool(name="e_sel", bufs=1) as selpool, \
                 tc.tile_pool(name="e_hd", bufs=3) as hdpool, \
                 tc.tile_pool(name="e_x", bufs=3) as xppool, \
                 tc.tile_pool(name="e_w", bufs=2) as wpool2, \
                 tc.tile_pool(name="e_psw", bufs=2, space="PSUM") as pswin, \
                 tc.tile_pool(name="e_psse", bufs=1, space="PSUM") as pssse, \
                 tc.tile_pool(name="e_psfin", bufs=1, space="PSUM") as psfin:
              for p in range(NSEM):
                for w in range(NWIN):
                    nb = float(w * 128)
                    idxt = stpool.tile([128, CALLS], I32, tag="idxt")
                    nc.sync.dma_start(idxt[:], gidx[p, w, :, :])
                    sct = stpool.tile([128, CALLS], F32, tag="sct")
                    nc.sync.dma_start(sct[:], scol[p, w, :, :])
                    srt = stpool.tile([128, CALLS * 128], F32, tag="srt")
                    sr1 = srow[p, w, :]
                    nc.sync.dma_start(
                        srt[:],
                        bass.AP(sr1.tensor, sr1.offset, [[0, 128], [1, CALLS * 128]]))
                    ssw = stpool.tile([128, 8], F32, tag="ssw")
                    nc.sync.dma_start(ssw[:], ssrc[p, w * 128:(w + 1) * 128, :])
                    ncol = stpool.tile([128, 1], F32, tag="ncol")
                    nc.vector.tensor_scalar_add(ncol[:], icol[:], nb)
                    nrow = stpool.tile([128, 128], F32, tag="nrow")
                    nc.vector.tensor_scalar_add(nrow[:], irow[:], nb)

                    ST = selpool.tile([128, CALLS * 128], F32, tag="ST")
                    ST3 = ST[:].rearrange("p (c e) -> p c e", c=CALLS)
                    nc.vector.tensor_tensor(
                        ST3,
                        _apx(sct[:], None, [1, CALLS], [0, 128]),
                        _apx(nrow[:], None, [0, CALLS], [1, 128]),
                        op=mybir.AluOpType.is_equal)
                    S = selpool.tile([128, CALLS * 128], F32, tag="S")
                    S3 = S[:].rearrange("p (c e) -> p c e", c=CALLS)
                    nc.vector.tensor_tensor(
                        S3,
                        _apx(ncol[:], None, [0, CALLS], [0, 128]),
                        _apx(srt[:], None, [128, CALLS], [1, 128]),
                        op=mybir.AluOpType.is_equal)

                    psA = pswin.tile([128, D], F32, tag="psA")
                    psB = pswin.tile([128, 8], F32, tag="psB")
                    for blk in range(NBLK):
                        cb = min(CB, CALLS - blk * CB)
                        c0 = blk * CB
                        hd = hdpool.tile([128, cb * TC], F32, tag="hd")
                        sse = pssse.tile([128, cb * 8], F32, tag="sse")
                        for ci in range(cb):
                            c = c0 + ci
                            nc.gpsimd.indirect_dma_start(
                                out=hd[:, ci * TC:(ci + 1) * TC],
                                out_offset=None,
                                in_=G[p][:, :],
                                in_offset=IndirectOffsetOnAxis(ap=idxt[:, c:c + 1], axis=0),
                            )
                            nc.tensor.matmul(sse[:, ci * 8:(ci + 1) * 8],
                                             lhsT=S3[:, c, :], rhs=ssw[:],
                                             start=True, stop=True)
                        hd3 = hd[:].rearrange("p (c f) -> p c f", c=cb)
                        wv = wpool2.tile([128, cb * 8], F32, tag="wv")
                        wv3 = wv[:].rearrange("p (c h) -> p c h", c=cb)
                        nc.vector.tensor_tensor(wv3, sse[:].rearrange("p (c h) -> p c h", c=cb),
                                                hd3[:, :, D:D + 8], op=mybir.AluOpType.add)
                        tv = wpool2.tile([128, cb * 8], F32, tag="tv")
                        nc.vector.tensor_scalar_mul(tv[:], wv[:], ALPHA)
                        nc.vector.tensor_tensor(wv[:], wv[:], tv[:], op=mybir.AluOpType.max)
                        nc.scalar.activation(wv[:], wv[:], mybir.ActivationFunctionType.Exp,
                                             scale=-1.0)
                        xb = xppool.tile([128, cb * D], F32, tag="xb")
                        nc.vector.tensor_tensor(
                            xb[:].rearrange("p (c h d) -> p c h d", c=cb, h=8),
                            hd3[:, :, 0:D].rearrange("p c (h d) -> p c h d", h=8),
                            wv3.to_broadcast([128, cb, 8, NHID]),
                            op=mybir.AluOpType.mult)
                        for ci in range(cb):
                            c = c0 + ci
                            nc.tensor.matmul(psA[:], lhsT=ST3[:, c, :],
                                             rhs=xb[:, ci * D:(ci + 1) * D],
                                             start=(c == 0), stop=(c == CALLS - 1))
                            nc.tensor.matmul(psB[:], lhsT=ST3[:, c, :],
                                             rhs=wv[:, ci * 8:(ci + 1) * 8],
                                             start=(c == 0), stop=(c == CALLS - 1))

                    # window finalize
                    den = zpool.tile([128, 8], F32, tag="den")
                    nc.vector.tensor_scalar_add(den[:], psB[:], 1e-16)
                    rec = zpool.tile([128, 8], F32, tag="rec")
                    nc.vector.reciprocal(rec[:], den[:])
                    zw = zpool.tile([128, D], F32, tag="zw")
                    nc.vector.tensor_tensor(
                        zw[:].rearrange("p (h d) -> p h d", h=8),
                        psA[:].rearrange("p (h d) -> p h d", h=8),
                        _apx(rec[:], None, [1, 8], [0, NHID]),
                        op=mybir.AluOpType.mult)
                    ze = zpool.tile([128, D], F32, tag="ze")
                    nc.vector.tensor_scalar_min(ze[:], zw[:], 0.0)
                    nc.scalar.activation(ze[:], ze[:], mybir.ActivationFunctionType.Exp)
                    nc.vector.tensor_scalar_add(ze[:], ze[:], -1.0)
                    nc.vector.tensor_tensor(zw[:], zw[:], ze[:], op=mybir.AluOpType.max)

                    pzw = psfin.tile([128, 128], F32, tag="pzw")
                    for k in range(4):
                        tp = psfin.tile([128, 128], F32, tag="tp")
                        nc.tensor.transpose(tp[:], zw[:, k * 128:(k + 1) * 128], ident[:])
                        zk = zpool.tile([128, 128], F32, tag="zk")
                        nc.vector.tensor_copy(zk[:], tp[:])
                        nc.sync.dma_start(zT[p, k, :, w * 128:(w + 1) * 128], zk[:])
                        nc.tensor.matmul(pzw[:], lhsT=wp_sb[:, k * NMP:(k + 1) * NMP],
                                         rhs=zk[:], start=(k == 0), stop=(k == 3))
                    tnh = zpool.tile([128, 128], F32, tag="tnh")
                    nc.scalar.activation(tnh[:], pzw[:], mybir.ActivationFunctionType.Tanh,
                                         bias=bp_sb[:, 0:1])
                    psq = psfin.tile([1, 128], F32, tag="psq")
                    nc.tensor.matmul(psq[:], lhsT=q_sb[:], rhs=tnh[:], start=True, stop=True)
                    nc.vector.tensor_add(wacc[p][:], wacc[p][:], psq[:])

            # ---------------- Phase W: beta via AllReduce ---------------------
            ws2 = accpool.tile([1, NSEM], F32, tag="ws2")
            for p in range(NSEM):
                nc.vector.reduce_sum(ws2[:, p:p + 1], wacc[p][:], axis=mybir.AxisListType.X)
            wb_sb = accpool.tile([1, NSEM], F32, tag="wb")
            nc.sync.dma_start(wb_sb[:], wbias[:, :])
            nc.vector.tensor_add(ws2[:], ws2[:], wb_sb[:])
            tc.strict_bb_all_engine_barrier()
            nc.sync.dma_start(wsin[:, :], ws2[:])
            tc.strict_bb_all_engine_barrier()
            nc.gpsimd.collective_compute(
                "AllReduce", mybir.AluOpType.add,
                replica_groups=[list(range(NCORES))],
                ins=[wsin[:, :]], outs=[wsout[:, :]])
            tc.strict_bb_all_engine_barrier()
            wsr = accpool.tile([1, NSEM], F32, tag="wsr")
            nc.sync.dma_start(wsr[:], wsout[:, :])
            nc.vector.tensor_scalar_mul(wsr[:], wsr[:], 1.0 / N)
            nc.scalar.activation(wsr[:], wsr[:], mybir.ActivationFunctionType.Exp)
            ssum = accpool.tile([1, 1], F32, tag="ssum")
            nc.vector.reduce_sum(ssum[:], wsr[:], axis=mybir.AxisListType.X)
            rsum = accpool.tile([1, 1], F32, tag="rsum")
            nc.vector.reciprocal(rsum[:], ssum[:])
            beta = accpool.tile([1, NSEM], F32, tag="beta")
            nc.vector.tensor_scalar_mul(beta[:], wsr[:], rsum[:, 0:1])
            ones = accpool.tile([1, 128], F32, tag="ones")
            nc.vector.memset(ones[:], 1.0)
            psf2 = tc.tile_pool(name="f_ps", bufs=2, space="PSUM")
            psfin2 = psf2.__enter__()
            psbt = psfin2.tile([128, NSEM], F32, tag="psbt")
            nc.tensor.matmul(psbt[:], lhsT=ones[:], rhs=beta[:], start=True, stop=True)
            bsb = accpool.tile([128, NSEM], F32, tag="bsb")
            nc.vector.tensor_copy(bsb[:], psbt[:])
            wc_sb = accpool.tile([128, 4 * NLABEL], F32, tag="wc")
            for k in range(4):
                nc.sync.dma_start(wc_sb[:, k * NLABEL:(k + 1) * NLABEL],
                                  Wc[k * 128:(k + 1) * 128, :])

            # ---------------- Phase F: combine + classifier -------------------
            for w in range(NWIN):
                pso = psfin2.tile([NLABEL, 128], F32, tag="pso")
                for k in range(4):
                    z0 = zpool.tile([128, 128], F32, tag="z0")
                    nc.sync.dma_start(z0[:], zT[0, k, :, w * 128:(w + 1) * 128])
                    z1 = zpool.tile([128, 128], F32, tag="z1")
                    nc.sync.dma_start(z1[:], zT[1, k, :, w * 128:(w + 1) * 128])
                    fk = zpool.tile([128, 128], F32, tag="fk")
                    nc.vector.tensor_scalar_mul(fk[:], z0[:], bsb[:, 0:1])
                    nc.vector.scalar_tensor_tensor(fk[:], z1[:], bsb[:, 1:2], fk[:],
                                                   op0=mybir.AluOpType.mult,
                                                   op1=mybir.AluOpType.add)
                    nc.tensor.matmul(pso[:], lhsT=wc_sb[:, k * NLABEL:(k + 1) * NLABEL],
                                     rhs=fk[:], start=(k == 0), stop=(k == 3))
                sg = zpool.tile([NLABEL, 128], F32, tag="sg")
                nc.scalar.activation(sg[:], pso[:], mybir.ActivationFunctionType.Sigmoid)
                nc.sync.dma_start(outT[:, w * 128:(w + 1) * 128], sg[:])
            psf2.__exit__(None, None, None)

    nc.compile()
    return nc


# ------------------------------------------------------------- host side ---
def _preprocess(x, adjs, W, a, Wp, bp, q, Wc):
    x = np.asarray(x, np.float32)
    adjs = np.asarray(adjs)
    W = np.asarray(W, np.float32)
    a = np.asarray(a, np.float32)
    Wp = np.asarray(Wp, np.float32)
    bp = np.asarray(bp, np.float32)
    q = np.asarray(q, np.float32)
    Wc = np.asarray(Wc, np.float32)

    xT = np.zeros((NFEAT, NPAD), np.float32)
    xT[:, :N] = x.T

    Waug = np.zeros((NSEM, NFEAT, TC), np.float32)
    for p in range(NSEM):
        Waug[p, :, :D] = W[p].transpose(1, 0, 2).reshape(NFEAT, D)
        Waug[p, :, D:D + 8] = np.einsum("hfd,hd->fh", W[p], a[p, :, NHID:])
        Waug[p, :, D + 8:] = np.einsum("hfd,hd->fh", W[p], a[p, :, :NHID])

    # per (path, core): sort edges by src, bucket into 128-node windows
    per_core = [[None] * NSEM for _ in range(NCORES)]
    maxcnt = 0
    for p in range(NSEM):
        src = np.asarray(adjs[p, 0], np.int64)
        dst = np.asarray(adjs[p, 1], np.int64)
        order = np.argsort(src, kind="stable")
        src_s, dst_s = src[order], dst[order]
        bounds = np.searchsorted(src_s, np.arange(NCORES + 1) * NPC)
        for c in range(NCORES):
            lo, hi = bounds[c], bounds[c + 1]
            ls = (src_s[lo:hi] - c * NPC).astype(np.int64)
            ld = dst_s[lo:hi]
            wid = ls >> 7
            cnt = np.bincount(wid, minlength=NWIN)
            maxcnt = max(maxcnt, int(cnt.max()))
            per_core[c][p] = (ls, ld, wid, cnt)
    CALLS = (maxcnt + 127) // 128
    MAXW = CALLS * 128

    gidx = np.full((NCORES, NSEM, NWIN, MAXW), DUMMY, np.int32)
    scol = np.zeros((NCORES, NSEM, NWIN, MAXW), np.float32)
    for c in range(NCORES):
        for p in range(NSEM):
            ls, ld, wid, cnt = per_core[c][p]
            offs = np.zeros(NWIN, np.int64)
            offs[1:] = np.cumsum(cnt)[:-1]
            rank = np.arange(ls.shape[0], dtype=np.int64) - offs[wid]
            scol[c, p][:, :] = (np.arange(NWIN, dtype=np.float32) * 128)[:, None]
            gidx[c, p, wid, rank] = ld.astype(np.int32)
            scol[c, p, wid, rank] = ls.astype(np.float32)
    # blocked layouts: [win, 128, CALLS] (col/idx tiles) and [win, CALLS*128]
    g4 = gidx.reshape(NCORES, NSEM, NWIN, CALLS, 128)
    s4 = scol.reshape(NCORES, NSEM, NWIN, CALLS, 128)
    gidx_t = np.ascontiguousarray(g4.transpose(0, 1, 2, 4, 3))
    scol_t = np.ascontiguousarray(s4.transpose(0, 1, 2, 4, 3))
    srow = np.ascontiguousarray(scol.reshape(NCORES, NSEM, NWIN, MAXW))

    phi = float(np.tanh(bp) @ q)
    wb = np.full((1, NSEM), -(NPC_PAD - NPC) * phi, np.float32)

    in_maps = []
    for c in range(NCORES):
        xTc = np.zeros((NFEAT, NPC_PAD), np.float32)
        xTc[:, :NPC] = x[c * NPC:(c + 1) * NPC].T
        in_maps.append({
            "xT": xT, "xTc": xTc, "Waug": Waug,
            "gidx": gidx_t[c], "scol": scol_t[c], "srow": srow[c],
            "Wp": Wp, "bp": bp.reshape(NMP, 1), "qv": q.reshape(NMP, 1),
            "Wc": Wc, "wbias": wb,
        })
    return in_maps, CALLS


_PROG_CACHE = {}


def kernel(x, adjs, W, a, Wp, bp, q, Wc, _trace=False):
    in_maps, CALLS = _preprocess(x, adjs, W, a, Wp, bp, q, Wc)
    if CALLS not in _PROG_CACHE:
        _PROG_CACHE[CALLS] = build_program(CALLS)
    nc = _PROG_CACHE[CALLS]
    try:
        res = run_bass_kernel_spmd(nc, in_maps, core_ids=list(range(NCORES)),
                                   trace=_trace)
    except ModuleNotFoundError:
        res = run_bass_kernel_spmd(nc, in_maps, core_ids=list(range(NCORES)),
                                   trace=False)
    out = np.concatenate(
        [res.results[c]["outT"].T[:NPC] for c in range(NCORES)], axis=0)
    if _trace:
        kernel.last_results = res
    return out

